# revision 21
# baseline (speedup 1.0000x reference)
"""NodeAttention Trainium2 kernel (per-core program, SPMD over 8 cores).

Strategy (per core, i-block of NI=96 query rows):
- host pre-arranges the core's pair slice as pairT [c, jb, i, j] bf16 so the
  device does one contiguous full-bandwidth DMA per j-block (24.6KB per
  partition) -- no SWDGE gather, no SBUF transposes.
- pair pools (SBUF TP buffers + dots/ss PSUM banks) open BEFORE the node
  section so TP prefetch and the jb0 projection work overlap node compute.
- pair LN + bias projection folded into a [128 chan -> 9] matmul:
  cols 0-7: lnw*w_bias - s_h/C (so dot' = dot - s_h*mu), col 8: 1/C (mean).
  bias_h = r*dot'_h + t_h with r = rsqrt(var+eps), var = sumsq/C - mu^2,
  sumsq via ones-matmul on squared tiles (squares split scalar/vector;
  scalar groups SQUARE ops to avoid activation-table thrash).
- t_h enters through an augmented 33rd contraction row of the q/k matmul
  (kT row 32 = 1, qT row 32 = t_h), so logits = sim' + r*dot' in one add.
- softmax without max-subtraction (logits bounded); normalizer via a ones
  column appended to V, so no partition reductions are needed.
"""
import numpy as np
from contextlib import ExitStack

import concourse.bass as bass
import concourse.tile as tile
from concourse import mybir
from concourse.masks import make_identity

f32 = mybir.dt.float32
bf16 = mybir.dt.bfloat16
u8 = mybir.dt.uint8

N = 768          # sequence length (j axis, also full i)
C = 128          # pair channels
H = 8            # heads
D = 32           # head dim
INNER = 256      # H*D
ND = 256         # node dim
NJB = N // 128   # 6 j-blocks
EPS = 1e-5


def _bcast_h(ap2d: bass.AP, h: int) -> bass.AP:
    """[P, F] -> [P, h, F] with step-0 broadcast over the middle dim."""
    ap = list(ap2d.ap)
    assert len(ap) == 2
    return bass.AP(ap2d.tensor, ap2d.offset, [ap[0], [0, h], ap[1]])


def _swap_hi(ap3: bass.AP, i_off: int, i_cnt: int) -> bass.AP:
    """logits [P, H, NI] tile -> iteration [P, i_cnt, H] at i offset."""
    p, hdim, idim = ap3.ap
    return bass.AP(ap3.tensor, ap3.offset + i_off * idim[0],
                   [p, [idim[0], i_cnt], hdim])


def _bcast_last(ap2d: bass.AP, i_off: int, i_cnt: int, h: int) -> bass.AP:
    """r [P, NI] -> iteration [P, i_cnt, h(step0)] at i offset."""
    p, f = ap2d.ap
    return bass.AP(ap2d.tensor, ap2d.offset + i_off * f[0],
                   [p, [f[0], i_cnt], [0, h]])


def build_nc(NI=96, n_cores=8, upto='full', trivial_lnb=True):
    nc = bass.Bass("TRN2", target_bir_lowering=False, debug=False,
                   num_devices=n_cores)
    # pair slice pre-transposed on host: pairT[c, jb, i, j] bf16
    pairT = nc.dram_tensor("pairT", [C, NJB, NI, 128], bf16,
                           kind="ExternalInput").ap()
    node = nc.dram_tensor("node", [N, ND], f32, kind="ExternalInput").ap()
    nodeq = nc.dram_tensor("nodeq", [NI, ND], f32, kind="ExternalInput").ap()
    m01 = nc.dram_tensor("m01", [128, NJB, NI], bf16, kind="ExternalInput").ap()
    wext = nc.dram_tensor("wext", [C, 16], bf16, kind="ExternalInput").ap()
    # wnode cols: [Wq*scale | Wk | Wv | Wg]
    wnode = nc.dram_tensor("wnode", [ND, 4 * INNER], bf16, kind="ExternalInput").ap()
    wout = nc.dram_tensor("wout", [INNER, ND], bf16, kind="ExternalInput").ap()
    lnw = nc.dram_tensor("lnw", [1, ND], f32, kind="ExternalInput").ap()
    lnb = nc.dram_tensor("lnb", [1, ND], f32, kind="ExternalInput").ap()
    bg = nc.dram_tensor("bg", [1, INNER], f32, kind="ExternalInput").ap()
    bout = nc.dram_tensor("bout", [1, ND], f32, kind="ExternalInput").ap()
    # t_h replicated over i (augmented q row)
    threp = nc.dram_tensor("threp", [1, H, NI], f32, kind="ExternalInput").ap()
    y_out = nc.dram_tensor("y", [NI, ND], f32, kind="ExternalOutput").ap()
    dbg = nc.dram_tensor("dbg", [128, 2048], f32, kind="ExternalOutput").ap() \
        if upto != 'full' else None

    NIT = (NI + 31) // 32          # dots psum tiles per jb

    with tile.TileContext(nc) as tc, ExitStack() as ctx:
        const = ctx.enter_context(tc.tile_pool(name="const", bufs=1))
        persist = ctx.enter_context(tc.tile_pool(name="persist", bufs=1))
        # pair-path pools, opened early so TP prefetch + jb0 dots/ss overlap
        # the node section (PSUM: dots 3 banks + ss 1 bank; node uses 4).
        jw = ctx.enter_context(tc.tile_pool(name="jwork", bufs=2))
        accp = ctx.enter_context(tc.tile_pool(name="att_acc", bufs=1))
        pctx = ctx.enter_context(ExitStack())
        dps = pctx.enter_context(tc.tile_pool(name="dots_ps", bufs=NIT, space="PSUM"))
        ssp = pctx.enter_context(tc.tile_pool(name="ss_ps", bufs=1, space="PSUM"))
        tpp = pctx.enter_context(tc.tile_pool(name="tp", bufs=4))
        sqp = pctx.enter_context(tc.tile_pool(name="tpsq", bufs=3))

        # ---- constants ----
        wext_sb = const.tile([C, 16], bf16)
        nc.scalar.dma_start(out=wext_sb[:], in_=wext)
        ones_sb = const.tile([C, 1], bf16)
        nc.vector.memset(ones_sb[:], 1.0)
        ident = const.tile([128, 128], f32)
        make_identity(nc, ident[:])
        eps_sb = const.tile([128, 1], f32)
        nc.vector.memset(eps_sb[:], EPS)
        # broadcast loads (replicate along partitions via step-0 DMA)
        def bload(name, src, cols, dtype=f32):
            t = const.tile([128, cols], dtype)
            src_b = bass.AP(src.tensor, src.offset, [[0, 128]] + list(src.ap)[1:])
            nc.gpsimd.dma_start(out=t[:], in_=src_b)
            return t
        lnb_sb = bload("lnb", lnb, ND)
        bg_sb = bload("bg", bg, INNER)
        bout_sb = bload("bout", bout, ND)
        # node-side weights, feat-major tiles [feat%128, feat//128, cols]
        wn_sb = const.tile([128, 2, 4 * INNER], bf16)
        nc.sync.dma_start(out=wn_sb[:],
                          in_=wnode.rearrange("(kt p) c -> p kt c", p=128))
        wout_sb = const.tile([128, 2, ND], bf16)
        nc.sync.dma_start(out=wout_sb[:],
                          in_=wout.rearrange("(kt p) c -> p kt c", p=128))

        # ---- persistent node-derived tensors (33rd row = t_h injection) ----
        kT_sb = persist.tile([33, H, N], bf16)        # k^T [d, h, j]; row 32 = 1
        qT_sb = persist.tile([33, H, NI], bf16)       # q^T [d, h, i]; row 32 = t_h
        Vx_sb = persist.tile([128, NJB, H, D + 1], bf16)  # v in [j, jb, h, d|1]
        m01T_sb = persist.tile([128, NJB, NI], bf16)  # mask^T in [j, jb, i]
        sig_sb = persist.tile([max(NI, 1), INNER], f32)  # sigmoid(g) [i, inner]

        nc.vector.memset(kT_sb[32:33, :, :], 1.0)
        nc.gpsimd.dma_start(out=qT_sb[32:33, :, :], in_=threp)  # f32 -> bf16 cast

        att_acc = accp.tile([NI, H, D + 1], f32)
        nc.vector.memset(att_acc[:], 0.0)

        # ---- pair-path stages (A: node-independent; B: needs k/q/V/mask) ----
        A = {}

        def stageA(jb):
            TP = TPs[jb]
            dots_tiles = []
            for t in range(NIT):
                dt_ = dps.tile([128, 32, 16], f32, tag="dots", name=f"dt{jb}_{t}")
                dots_tiles.append(dt_)
            for i in range(NI):
                nc.tensor.matmul(dots_tiles[i // 32][:, i % 32, 0:9],
                                 lhsT=TP[:, i, :], rhs=wext_sb[:, 0:9])
            # sumsq via squared tiles: chunk 0 scalar, 1-2 vector
            ss_ps = ssp.tile([128, NI], f32, tag="ss", name=f"ss{jb}")
            for t in range(NIT):
                i0 = t * 32
                tsq = sqp.tile([C, 32, 128], bf16, tag="tpsq", name=f"tsq{jb}_{t}")
                src = TP[:, i0:i0 + 32, :]
                if t < 2:
                    nc.scalar.square(tsq[:], src)
                else:
                    nc.vector.tensor_mul(tsq[:], src, src)
                for il in range(32):
                    nc.tensor.matmul(ss_ps[:, i0 + il:i0 + il + 1],
                                     lhsT=tsq[:, il, :], rhs=ones_sb[:])
            # early PSUM->SBUF copy of dots so the banks recycle immediately
            dots_sb = jw.tile([128, NI, 9], f32, tag="dots_sb", bufs=3,
                              name=f"dsb{jb}")
            for t in range(NIT):
                nc.vector.tensor_copy(dots_sb[:, t * 32:(t + 1) * 32, :],
                                      dots_tiles[t][:, :, 0:9])
            # stats: var = ss/C - mu^2 ; r = 1/sqrt(var + eps)
            m2 = jw.tile([128, NI], f32, tag="m2", bufs=3, name=f"m2_{jb}")
            nc.scalar.square(m2[:], dots_sb[:, :, 8])
            var = jw.tile([128, NI], f32, tag="var", bufs=3, name=f"var{jb}")
            nc.vector.scalar_tensor_tensor(
                out=var[:], in0=ss_ps[:], scalar=1.0 / C, in1=m2[:],
                op0=mybir.AluOpType.mult, op1=mybir.AluOpType.subtract)
            sdp = jw.tile([128, NI], f32, tag="sdp", bufs=3, name=f"sdp{jb}")
            nc.scalar.activation(sdp[:], var[:],
                                 mybir.ActivationFunctionType.Sqrt,
                                 bias=eps_sb[:])
            r = jw.tile([128, NI], f32, tag="r", bufs=3, name=f"r{jb}")
            nc.vector.reciprocal(r[:], sdp[:])
            A[jb] = (dots_sb, r)

        def stageB(jb):
            dots_sb, r = A.pop(jb)
            sim_ps = simp.tile([128, H, 128], f32, tag="sim", name=f"sim{jb}")
            for h in range(H):
                nc.tensor.matmul(
                    sim_ps[:, h, 0:NI],
                    lhsT=kT_sb[:, h, jb * 128:(jb + 1) * 128],
                    rhs=qT_sb[:, h, :])
            # logits = sim' + r*dots' ; E = exp * mask
            logits = jw.tile([128, H, NI], f32, tag="logits", bufs=2, name=f"lg{jb}")
            nc.vector.tensor_tensor(
                out=_swap_hi(logits[:], 0, NI),
                in0=dots_sb[:, :, 0:8],
                in1=_bcast_last(r[:], 0, NI, H),
                op=mybir.AluOpType.mult)
            nc.vector.tensor_add(logits[:], logits[:], sim_ps[:, :, 0:NI])
            E = jw.tile([128, H, NI], bf16, tag="E", bufs=2, name=f"E{jb}")
            nc.scalar.activation(E[:], logits[:],
                                 mybir.ActivationFunctionType.Exp)
            nc.vector.tensor_mul(E[:], E[:], _bcast_h(m01T_sb[:, jb, :], H))
            # attn @ [V|1]; accumulate across jb in SBUF
            att_ps = attp.tile([NI, H, D + 1], f32, tag="attjb", name=f"at{jb}")
            for h in range(H):
                nc.tensor.matmul(att_ps[:, h, :], lhsT=E[:, h, :],
                                 rhs=Vx_sb[:, jb, h, :])
            nc.vector.tensor_add(att_acc[:], att_acc[:], att_ps[:])

        with tc.tile_pool(name="nodep", bufs=1) as npool, \
             tc.tile_pool(name="node_ps", bufs=1, space="PSUM") as nps:
            # -- load + LN node (full, for k/v) and nodeq (for q/g) --
            x_all = npool.tile([128, N // 128, ND], f32)
            stats = npool.tile([128, 6], f32)
            mv = npool.tile([128, 2], f32)
            sd = npool.tile([128, 1], f32)
            rln = npool.tile([128, 1], f32)

            def layernorm_tile(xt, nrows):
                nc.vector.bn_stats(out=stats[:nrows, :], in_=xt)
                nc.vector.bn_aggr(out=mv[:nrows, :], in_=stats[:nrows, :])
                nc.scalar.activation(sd[:nrows, :], mv[:nrows, 1:2],
                                     mybir.ActivationFunctionType.Sqrt,
                                     bias=eps_sb[:nrows, :])
                nc.vector.reciprocal(rln[:nrows, :], sd[:nrows, :])
                nc.vector.tensor_scalar(out=xt, in0=xt,
                                        scalar1=mv[:nrows, 0:1],
                                        scalar2=rln[:nrows, :],
                                        op0=mybir.AluOpType.subtract,
                                        op1=mybir.AluOpType.mult)
                if not trivial_lnb:
                    nc.vector.tensor_add(xt, xt, lnb_sb[:nrows, :])

            # node input loads go on the scalar HWDGE queue so they don't
            # wait behind the 12MB TP bulk on the sync queue.
            for t in range(N // 128):
                nc.scalar.dma_start(out=x_all[:, t, :], in_=node[t * 128:(t + 1) * 128, :])
            xq = npool.tile([NI, ND], f32)
            nc.scalar.dma_start(out=xq[:], in_=nodeq)
            nc.scalar.dma_start(out=m01T_sb[:], in_=m01)
            TPs = []
            for jb in range(NJB):
                TP = tpp.tile([C, NI, 128], bf16, tag="tp", name=f"TP{jb}")
                nc.sync.dma_start(out=TP[:], in_=pairT[:, jb, :, :])
                TPs.append(TP)

            for t in range(N // 128):
                layernorm_tile(x_all[:, t, :], 128)
            layernorm_tile(xq[:], NI)

            # jb0/jb1 projection work is node-independent: emit it here so
            # PE/vector/scalar have pair work queued while node copies drain.
            stageA(0)
            stageA(1)

            # -- transposes: xT [feat, j], xqT [feat, i] --
            xT_sb = npool.tile([128, 2, N], bf16)
            xqT_sb = npool.tile([128, 2, NI], bf16)
            for t in range(N // 128):
                tp = nps.tile([128, 2, 128], f32, tag="xpose", bufs=1)
                for kt in range(2):
                    nc.tensor.transpose(tp[:, kt, :], x_all[:, t, kt * 128:(kt + 1) * 128], ident[:])
                nc.vector.tensor_copy(xT_sb[:, :, t * 128:(t + 1) * 128], tp[:])
            tp = nps.tile([128, 2, 128], f32, tag="xpose", bufs=1)
            for kt in range(2):
                nc.tensor.transpose(tp[:, kt, :NI], xq[:, kt * 128:(kt + 1) * 128], ident[:NI, :NI])
            nc.vector.tensor_copy(xqT_sb[:, :, :], tp[:, :, :NI])

            # -- k^T = Wk^T @ x^T : per-head [d, j] at partition base 0 --
            for h in range(H):
                for n0 in range(0, N, 384):
                    kp = nps.tile([128, 384], f32, tag="nmm", bufs=2)
                    for kt in range(2):
                        nc.tensor.matmul(
                            kp[:32, :],
                            lhsT=wn_sb[:, kt, INNER + h * D:INNER + (h + 1) * D],
                            rhs=xT_sb[:, kt, n0:n0 + 384],
                            start=(kt == 0), stop=(kt == 1))
                    nc.vector.tensor_copy(kT_sb[:32, h, n0:n0 + 384], kp[:32, :])

            # -- v = x @ Wv -> Vx [j, jb, h, d] + ones column --
            for jb in range(NJB):
                vp = nps.tile([128, 384], f32, tag="nmm", bufs=2)
                for kt in range(2):
                    nc.tensor.matmul(vp[:, 0:INNER], lhsT=xT_sb[:, kt, jb * 128:(jb + 1) * 128],
                                     rhs=wn_sb[:, kt, 2 * INNER:3 * INNER],
                                     start=(kt == 0), stop=(kt == 1))
                nc.vector.tensor_copy(Vx_sb[:, jb, :, 0:D],
                                      vp[:, 0:INNER].rearrange("p (h d) -> p h d", h=H))
            nc.vector.memset(Vx_sb[:, :, :, D:D + 1], 1.0)

            # -- q^T = (Wq*scale)^T @ xq^T : per-head [d, i] --
            for h in range(H):
                qp = nps.tile([128, 384], f32, tag="nmm", bufs=2)
                for kt in range(2):
                    nc.tensor.matmul(qp[:32, 0:NI], lhsT=wn_sb[:, kt, h * D:(h + 1) * D],
                                     rhs=xqT_sb[:, kt, :],
                                     start=(kt == 0), stop=(kt == 1))
                nc.vector.tensor_copy(qT_sb[:32, h, :], qp[:32, 0:NI])

            # -- g = xq @ Wg + bg ; sig = sigmoid(g) --
            gp = nps.tile([128, 384], f32, tag="nmm", bufs=2)
            for kt in range(2):
                nc.tensor.matmul(gp[:NI, 0:INNER], lhsT=xqT_sb[:, kt, :],
                                 rhs=wn_sb[:, kt, 3 * INNER:4 * INNER],
                                 start=(kt == 0), stop=(kt == 1))
            gt = npool.tile([NI, INNER], f32)
            nc.vector.tensor_add(gt[:], gp[:NI, 0:INNER], bg_sb[:NI, :])
            nc.scalar.activation(sig_sb[:NI, :], gt[:],
                                 mybir.ActivationFunctionType.Sigmoid)


        # ================= pair path (jb loop) =================
        with tc.tile_pool(name="sim_ps", bufs=1, space="PSUM") as simp, \
             tc.tile_pool(name="att_ps", bufs=2, space="PSUM") as attp:
            for jb in range(2, NJB):
                stageA(jb)
                stageB(jb - 2)
            stageB(NJB - 2)
            stageB(NJB - 1)

        pctx.close()   # release TP/tsq SBUF + dots/ss PSUM before finalize
        # ---- finalize ----
        if True:
            with tc.tile_pool(name="fin", bufs=1) as fin, \
                 tc.tile_pool(name="fin_ps", bufs=2, space="PSUM") as finp:
                den_r = fin.tile([NI, H], f32)
                for h in range(H):
                    nc.vector.reciprocal(den_r[:, h:h + 1], att_acc[:, h, D:D + 1])
                att_f = fin.tile([NI, INNER], f32)
                for h in range(H):
                    nc.vector.tensor_scalar_mul(att_f[:, h * D:(h + 1) * D],
                                                att_acc[:, h, 0:D],
                                                den_r[:, h:h + 1])
                gated = fin.tile([NI, INNER], f32)
                nc.vector.tensor_mul(gated[:], att_f[:], sig_sb[:NI, :])

                gT_sb = fin.tile([128, 2, NI], bf16)
                for kt in range(2):
                    tp = finp.tile([128, NI], f32, tag="gpose")
                    nc.tensor.transpose(tp[:], gated[:, kt * 128:(kt + 1) * 128],
                                        ident[:NI, :NI])
                    nc.vector.tensor_copy(gT_sb[:, kt, :], tp[:])

                y_ps = finp.tile([NI, ND], f32, tag="ymm")
                for kt in range(2):
                    nc.tensor.matmul(y_ps[:], lhsT=gT_sb[:, kt, :],
                                     rhs=wout_sb[:, kt, :],
                                     start=(kt == 0), stop=(kt == 1))
                y_sb = fin.tile([NI, ND], f32)
                nc.vector.tensor_add(y_sb[:], y_ps[:], bout_sb[:NI, :])
                nc.sync.dma_start(out=y_out, in_=y_sb[:])

    return nc


def host_prep(inputs, NI=96, n_cores=8):
    """Slice/fold FULL inputs into per-core in_maps."""
    import ml_dtypes
    bf = ml_dtypes.bfloat16
    node_feats = np.asarray(inputs["node_feats"])[0]      # [N, ND]
    pair_feats = np.asarray(inputs["pair_feats"])[0]      # [N, N, C]
    mask = np.asarray(inputs["mask"])[0]                  # [N, N] bool
    lnw = np.asarray(inputs["ln_node_w"]).reshape(1, ND)
    lnb = np.asarray(inputs["ln_node_b"]).reshape(1, ND)
    lpw = np.asarray(inputs["ln_pair_w"])                 # [C]
    lpb = np.asarray(inputs["ln_pair_b"])                 # [C]
    w_qkv = np.asarray(inputs["w_qkv"])                   # [ND, 3*INNER]
    w_g = np.asarray(inputs["w_g"])                       # [ND, INNER]
    b_g = np.asarray(inputs["b_g"]).reshape(1, INNER)
    w_bias = np.asarray(inputs["w_bias"])                 # [C, H]
    w_out = np.asarray(inputs["w_out"])                   # [INNER, ND]
    b_out = np.asarray(inputs["b_out"]).reshape(1, ND)

    Wp = lpw[:, None] * w_bias                            # [C, H]
    s_h = Wp.sum(0)
    t_h = (lpb[:, None] * w_bias).sum(0)
    wext = np.zeros((C, 16), np.float32)
    wext[:, 0:H] = Wp - s_h[None, :] / C                  # dot' = dot - s_h*mu
    wext[:, 8] = 1.0 / C
    wext = wext.astype(bf)

    scale = D ** -0.5
    # ln_node_w folded into the node-side weights (rows are feature-indexed)
    wnode = (lnw.reshape(ND, 1) *
             np.concatenate([w_qkv[:, 0:INNER] * scale,
                             w_qkv[:, INNER:2 * INNER],
                             w_qkv[:, 2 * INNER:3 * INNER],
                             w_g], axis=1)).astype(bf)
    woutb = w_out.astype(bf)
    threp = np.ascontiguousarray(
        np.broadcast_to(t_h[None, :, None], (1, H, NI))).astype(np.float32)

    shared = dict(node=node_feats.astype(np.float32), wext=wext, wnode=wnode,
                  wout=woutb, lnw=lnw.astype(np.float32), lnb=lnb.astype(np.float32),
                  bg=b_g.astype(np.float32), bout=b_out.astype(np.float32),
                  threp=threp)
    in_maps = []
    for c in range(n_cores):
        i0 = c * NI
        slab = pair_feats[i0:i0 + NI]                     # [NI, N, C] f32
        pairT = slab.transpose(2, 0, 1).reshape(C, NI, NJB, 128) \
                    .transpose(0, 2, 1, 3).astype(bf)     # [C, NJB, NI, 128]
        # mask pre-transposed to the kernel's [j%128, jb, i] layout
        m01 = mask[i0:i0 + NI].T.reshape(NJB, 128, NI) \
                  .transpose(1, 0, 2).astype(bf)
        in_maps.append(dict(
            pairT=np.ascontiguousarray(pairT),
            m01=np.ascontiguousarray(m01),
            nodeq=np.ascontiguousarray(node_feats[i0:i0 + NI]).astype(np.float32),
            **shared))
    return in_maps


def split_sync_waits(nc, limit=1):
    """Walrus (this container's neuronxcc) rejects instructions carrying more
    than `limit` sem waits. Hoist excess waits onto per-engine carrier drains
    inserted just before the offending instruction."""
    n_split = 0
    for f in nc.m.functions:
        for bb in f.blocks:
            out = []
            for inst in bb.instructions:
                si = inst.sync_info
                waits = list(si.on_wait) if si and si.on_wait else []
                if len(waits) > limit:
                    extra, keep = waits[:-limit], waits[-limit:]
                    for ci in range(0, len(extra), limit):
                        chunk = extra[ci:ci+limit]
                        nd = mybir.InstDrain(name=f"{inst.name}-wsplit{ci}", ins=[], outs=[])
                        nd.engine = inst.engine
                        nd.sync_info = mybir.SyncInfo(on_wait=chunk, on_update=[])
                        out.append(nd)
                        n_split += 1
                    si.on_wait = keep
                out.append(inst)
            bb.instructions = out
    return n_split


_CACHED = {}


def kernel(**inputs):
    """Full-input entry point: shards over 8 NeuronCores, returns full output."""
    NC_CORES = 8
    NI = N // NC_CORES
    from concourse.bass_utils import run_bass_kernel_spmd

    in_maps = host_prep(inputs, NI=NI, n_cores=NC_CORES)
    trivial_lnb = not np.any(np.asarray(inputs["ln_node_b"]))
    key = ("nc", trivial_lnb)
    if key not in _CACHED:
        nc = build_nc(NI=NI, n_cores=NC_CORES, trivial_lnb=trivial_lnb)
        split_sync_waits(nc)
        _CACHED[key] = nc
    res = run_bass_kernel_spmd(_CACHED[key], in_maps, list(range(NC_CORES)))
    y = np.concatenate([res.results[c]["y"] for c in range(NC_CORES)], axis=0)
    return y[None].astype(np.float32)


# revision 22
# speedup vs baseline: 1.0481x; 1.0481x over previous
"""NodeAttention Trainium2 kernel (per-core program, SPMD over 8 cores).

Strategy (per core, i-block of NI=96 query rows):
- host pre-arranges the core's pair slice as pairT [c, jb, i, j] bf16 so the
  device does one contiguous full-bandwidth DMA per j-block (24.6KB per
  partition) -- no SWDGE gather, no SBUF transposes.
- pair pools (SBUF TP buffers + dots/ss PSUM banks) open BEFORE the node
  section so TP prefetch and the jb0 projection work overlap node compute.
- pair LN + bias projection folded into a [128 chan -> 9] matmul:
  cols 0-7: lnw*w_bias - s_h/C (so dot' = dot - s_h*mu), col 8: 1/C (mean).
  bias_h = r*dot'_h + t_h with r = rsqrt(var+eps), var = sumsq/C - mu^2,
  sumsq via ones-matmul on squared tiles (squares split scalar/vector;
  scalar groups SQUARE ops to avoid activation-table thrash).
- t_h enters through an augmented 33rd contraction row of the q/k matmul
  (kT row 32 = 1, qT row 32 = t_h), so logits = sim' + r*dot' in one add.
- softmax without max-subtraction (logits bounded); normalizer via a ones
  column appended to V, so no partition reductions are needed.
"""
import numpy as np
from contextlib import ExitStack

import concourse.bass as bass
import concourse.tile as tile
from concourse import mybir
from concourse.masks import make_identity

f32 = mybir.dt.float32
bf16 = mybir.dt.bfloat16
u8 = mybir.dt.uint8

N = 768          # sequence length (j axis, also full i)
C = 128          # pair channels
H = 8            # heads
D = 32           # head dim
INNER = 256      # H*D
ND = 256         # node dim
NJB = N // 128   # 6 j-blocks
EPS = 1e-5


def _bcast_h(ap2d: bass.AP, h: int) -> bass.AP:
    """[P, F] -> [P, h, F] with step-0 broadcast over the middle dim."""
    ap = list(ap2d.ap)
    assert len(ap) == 2
    return bass.AP(ap2d.tensor, ap2d.offset, [ap[0], [0, h], ap[1]])


def _swap_hi(ap3: bass.AP, i_off: int, i_cnt: int) -> bass.AP:
    """logits [P, H, NI] tile -> iteration [P, i_cnt, H] at i offset."""
    p, hdim, idim = ap3.ap
    return bass.AP(ap3.tensor, ap3.offset + i_off * idim[0],
                   [p, [idim[0], i_cnt], hdim])


def _bcast_last(ap2d: bass.AP, i_off: int, i_cnt: int, h: int) -> bass.AP:
    """r [P, NI] -> iteration [P, i_cnt, h(step0)] at i offset."""
    p, f = ap2d.ap
    return bass.AP(ap2d.tensor, ap2d.offset + i_off * f[0],
                   [p, [f[0], i_cnt], [0, h]])


def build_nc(NI=96, n_cores=8, upto='full', trivial_lnb=True):
    nc = bass.Bass("TRN2", target_bir_lowering=False, debug=False,
                   num_devices=n_cores)
    # pair slice pre-transposed on host: pairT[c, jb, i, j] bf16
    pairT = nc.dram_tensor("pairT", [C, NJB, NI, 128], bf16,
                           kind="ExternalInput").ap()
    node = nc.dram_tensor("node", [N, ND], f32, kind="ExternalInput").ap()
    nodeq = nc.dram_tensor("nodeq", [NI, ND], f32, kind="ExternalInput").ap()
    m01 = nc.dram_tensor("m01", [128, NJB, NI], bf16, kind="ExternalInput").ap()
    wext = nc.dram_tensor("wext", [C, 16], bf16, kind="ExternalInput").ap()
    # wnode cols: [Wq*scale | Wk | Wv | Wg]
    wnode = nc.dram_tensor("wnode", [ND, 4 * INNER], bf16, kind="ExternalInput").ap()
    wout = nc.dram_tensor("wout", [INNER, ND], bf16, kind="ExternalInput").ap()
    lnw = nc.dram_tensor("lnw", [1, ND], f32, kind="ExternalInput").ap()
    lnb = nc.dram_tensor("lnb", [1, ND], f32, kind="ExternalInput").ap()
    bg = nc.dram_tensor("bg", [1, INNER], f32, kind="ExternalInput").ap()
    bout = nc.dram_tensor("bout", [1, ND], f32, kind="ExternalInput").ap()
    # t_h replicated over i (augmented q row)
    threp = nc.dram_tensor("threp", [1, H, NI], f32, kind="ExternalInput").ap()
    y_out = nc.dram_tensor("y", [NI, ND], f32, kind="ExternalOutput").ap()
    dbg = nc.dram_tensor("dbg", [128, 2048], f32, kind="ExternalOutput").ap() \
        if upto != 'full' else None

    NIT = (NI + 31) // 32          # dots psum tiles per jb

    with tile.TileContext(nc) as tc, ExitStack() as ctx:
        const = ctx.enter_context(tc.tile_pool(name="const", bufs=1))
        persist = ctx.enter_context(tc.tile_pool(name="persist", bufs=1))
        # pair-path pools, opened early so TP prefetch + jb0 dots/ss overlap
        # the node section (PSUM: dots 3 banks + ss 1 bank; node uses 4).
        jw = ctx.enter_context(tc.tile_pool(name="jwork", bufs=2))
        accp = ctx.enter_context(tc.tile_pool(name="att_acc", bufs=1))
        pctx = ctx.enter_context(ExitStack())
        dps = pctx.enter_context(tc.tile_pool(name="dots_ps", bufs=NIT, space="PSUM"))
        ssp = pctx.enter_context(tc.tile_pool(name="ss_ps", bufs=1, space="PSUM"))
        tpp = pctx.enter_context(tc.tile_pool(name="tp", bufs=4))
        sqp = pctx.enter_context(tc.tile_pool(name="tpsq", bufs=3))

        # ---- constants ----
        wext_sb = const.tile([C, 16], bf16)
        nc.scalar.dma_start(out=wext_sb[:], in_=wext)
        ones_sb = const.tile([C, 1], bf16)
        nc.vector.memset(ones_sb[:], 1.0)
        ident = const.tile([128, 128], f32)
        make_identity(nc, ident[:])
        eps_sb = const.tile([128, 1], f32)
        nc.vector.memset(eps_sb[:], EPS)
        # broadcast loads (replicate along partitions via step-0 DMA)
        def bload(name, src, cols, dtype=f32):
            t = const.tile([128, cols], dtype)
            src_b = bass.AP(src.tensor, src.offset, [[0, 128]] + list(src.ap)[1:])
            nc.gpsimd.dma_start(out=t[:], in_=src_b)
            return t
        lnb_sb = bload("lnb", lnb, ND)
        bg_sb = bload("bg", bg, INNER)
        bout_sb = bload("bout", bout, ND)
        # node-side weights, feat-major tiles [feat%128, feat//128, cols]
        wn_sb = const.tile([128, 2, 4 * INNER], bf16)
        nc.sync.dma_start(out=wn_sb[:],
                          in_=wnode.rearrange("(kt p) c -> p kt c", p=128))
        wout_sb = const.tile([128, 2, ND], bf16)
        nc.sync.dma_start(out=wout_sb[:],
                          in_=wout.rearrange("(kt p) c -> p kt c", p=128))

        # ---- persistent node-derived tensors (33rd row = t_h injection) ----
        kT_sb = persist.tile([33, H, N], bf16)        # k^T [d, h, j]; row 32 = 1
        qT_sb = persist.tile([33, H, NI], bf16)       # q^T [d, h, i]; row 32 = t_h
        Vx_sb = persist.tile([128, NJB, H, D + 1], bf16)  # v in [j, jb, h, d|1]
        m01T_sb = persist.tile([128, NJB, NI], bf16)  # mask^T in [j, jb, i]
        sig_sb = persist.tile([max(NI, 1), INNER], f32)  # sigmoid(g) [i, inner]

        nc.vector.memset(kT_sb[32:33, :, :], 1.0)
        nc.gpsimd.dma_start(out=qT_sb[32:33, :, :], in_=threp)  # f32 -> bf16 cast

        att_acc = accp.tile([NI, H, D + 1], f32)
        nc.vector.memset(att_acc[:], 0.0)

        # ---- pair-path stages (A: node-independent; B: needs k/q/V/mask) ----
        A = {}

        def stageA(jb):
            TP = TPs[jb]
            dots_tiles = []
            for t in range(NIT):
                dt_ = dps.tile([128, 32, 16], f32, tag="dots", name=f"dt{jb}_{t}")
                dots_tiles.append(dt_)
            for i in range(NI):
                nc.tensor.matmul(dots_tiles[i // 32][:, i % 32, 0:9],
                                 lhsT=TP[:, i, :], rhs=wext_sb[:, 0:9])
            # sumsq via squared tiles: chunk 0 scalar, 1-2 vector
            ss_ps = ssp.tile([128, NI], f32, tag="ss", name=f"ss{jb}")
            for t in range(NIT):
                i0 = t * 32
                tsq = sqp.tile([C, 32, 128], bf16, tag="tpsq", name=f"tsq{jb}_{t}")
                src = TP[:, i0:i0 + 32, :]
                if t == 0:
                    nc.scalar.square(tsq[:], src)
                else:
                    nc.vector.tensor_mul(tsq[:], src, src)
                for il in range(32):
                    nc.tensor.matmul(ss_ps[:, i0 + il:i0 + il + 1],
                                     lhsT=tsq[:, il, :], rhs=ones_sb[:])
            # early PSUM->SBUF copy of dots so the banks recycle immediately
            dots_sb = jw.tile([128, NI, 9], f32, tag="dots_sb", bufs=3,
                              name=f"dsb{jb}")
            for t in range(NIT):
                nc.vector.tensor_copy(dots_sb[:, t * 32:(t + 1) * 32, :],
                                      dots_tiles[t][:, :, 0:9])
            # stats: var = ss/C - mu^2 ; r = 1/sqrt(var + eps)
            m2 = jw.tile([128, NI], f32, tag="m2", bufs=3, name=f"m2_{jb}")
            nc.scalar.square(m2[:], dots_sb[:, :, 8])
            var = jw.tile([128, NI], f32, tag="var", bufs=3, name=f"var{jb}")
            nc.vector.scalar_tensor_tensor(
                out=var[:], in0=ss_ps[:], scalar=1.0 / C, in1=m2[:],
                op0=mybir.AluOpType.mult, op1=mybir.AluOpType.subtract)
            sdp = jw.tile([128, NI], f32, tag="sdp", bufs=3, name=f"sdp{jb}")
            nc.scalar.activation(sdp[:], var[:],
                                 mybir.ActivationFunctionType.Sqrt,
                                 bias=eps_sb[:])
            r = jw.tile([128, NI], f32, tag="r", bufs=3, name=f"r{jb}")
            nc.vector.reciprocal(r[:], sdp[:])
            A[jb] = (dots_sb, r)

        def stageB(jb):
            dots_sb, r = A.pop(jb)
            sim_ps = simp.tile([128, H, 128], f32, tag="sim", name=f"sim{jb}")
            for h in range(H):
                nc.tensor.matmul(
                    sim_ps[:, h, 0:NI],
                    lhsT=kT_sb[:, h, jb * 128:(jb + 1) * 128],
                    rhs=qT_sb[:, h, :])
            # logits = sim' + r*dots' ; E = exp * mask
            logits = jw.tile([128, H, NI], f32, tag="logits", bufs=2, name=f"lg{jb}")
            nc.vector.tensor_tensor(
                out=_swap_hi(logits[:], 0, NI),
                in0=dots_sb[:, :, 0:8],
                in1=_bcast_last(r[:], 0, NI, H),
                op=mybir.AluOpType.mult)
            nc.vector.tensor_add(logits[:], logits[:], sim_ps[:, :, 0:NI])
            E = jw.tile([128, H, NI], bf16, tag="E", bufs=2, name=f"E{jb}")
            nc.scalar.activation(E[:], logits[:],
                                 mybir.ActivationFunctionType.Exp)
            nc.vector.tensor_mul(E[:], E[:], _bcast_h(m01T_sb[:, jb, :], H))
            # attn @ [V|1]; accumulate across jb in SBUF
            att_ps = attp.tile([NI, H, D + 1], f32, tag="attjb", name=f"at{jb}")
            for h in range(H):
                nc.tensor.matmul(att_ps[:, h, :], lhsT=E[:, h, :],
                                 rhs=Vx_sb[:, jb, h, :])
            nc.vector.tensor_add(att_acc[:], att_acc[:], att_ps[:])

        with tc.tile_pool(name="nodep", bufs=1) as npool, \
             tc.tile_pool(name="node_ps", bufs=1, space="PSUM") as nps:
            # -- load + LN node (full, for k/v) and nodeq (for q/g) --
            x_all = npool.tile([128, N // 128, ND], f32)
            stats = npool.tile([128, 6], f32)
            mv = npool.tile([128, 2], f32)
            sd = npool.tile([128, 1], f32)
            rln = npool.tile([128, 1], f32)

            def layernorm_tile(xt, nrows):
                nc.vector.bn_stats(out=stats[:nrows, :], in_=xt)
                nc.vector.bn_aggr(out=mv[:nrows, :], in_=stats[:nrows, :])
                nc.scalar.activation(sd[:nrows, :], mv[:nrows, 1:2],
                                     mybir.ActivationFunctionType.Sqrt,
                                     bias=eps_sb[:nrows, :])
                nc.vector.reciprocal(rln[:nrows, :], sd[:nrows, :])
                nc.vector.tensor_scalar(out=xt, in0=xt,
                                        scalar1=mv[:nrows, 0:1],
                                        scalar2=rln[:nrows, :],
                                        op0=mybir.AluOpType.subtract,
                                        op1=mybir.AluOpType.mult)
                if not trivial_lnb:
                    nc.vector.tensor_add(xt, xt, lnb_sb[:nrows, :])

            # node input loads go on the scalar HWDGE queue so they don't
            # wait behind the 12MB TP bulk on the sync queue.
            for t in range(N // 128):
                nc.scalar.dma_start(out=x_all[:, t, :], in_=node[t * 128:(t + 1) * 128, :])
            xq = npool.tile([NI, ND], f32)
            nc.scalar.dma_start(out=xq[:], in_=nodeq)
            nc.scalar.dma_start(out=m01T_sb[:], in_=m01)
            TPs = []
            for jb in range(NJB):
                TP = tpp.tile([C, NI, 128], bf16, tag="tp", name=f"TP{jb}")
                nc.sync.dma_start(out=TP[:], in_=pairT[:, jb, :, :])
                TPs.append(TP)

            for t in range(N // 128):
                layernorm_tile(x_all[:, t, :], 128)
            layernorm_tile(xq[:], NI)

            # jb0/jb1 projection work is node-independent: emit it here so
            # PE/vector/scalar have pair work queued while node copies drain.
            stageA(0)
            stageA(1)

            # -- transposes: xT [feat, j], xqT [feat, i] --
            xT_sb = npool.tile([128, 2, N], bf16)
            xqT_sb = npool.tile([128, 2, NI], bf16)
            for t in range(N // 128):
                tp = nps.tile([128, 2, 128], f32, tag="xpose", bufs=1)
                for kt in range(2):
                    nc.tensor.transpose(tp[:, kt, :], x_all[:, t, kt * 128:(kt + 1) * 128], ident[:])
                nc.vector.tensor_copy(xT_sb[:, :, t * 128:(t + 1) * 128], tp[:])
            tp = nps.tile([128, 2, 128], f32, tag="xpose", bufs=1)
            for kt in range(2):
                nc.tensor.transpose(tp[:, kt, :NI], xq[:, kt * 128:(kt + 1) * 128], ident[:NI, :NI])
            nc.vector.tensor_copy(xqT_sb[:, :, :], tp[:, :, :NI])

            # -- k^T = Wk^T @ x^T : per-head [d, j] at partition base 0 --
            for h in range(H):
                for n0 in range(0, N, 384):
                    kp = nps.tile([128, 384], f32, tag="nmm", bufs=2)
                    for kt in range(2):
                        nc.tensor.matmul(
                            kp[:32, :],
                            lhsT=wn_sb[:, kt, INNER + h * D:INNER + (h + 1) * D],
                            rhs=xT_sb[:, kt, n0:n0 + 384],
                            start=(kt == 0), stop=(kt == 1))
                    nc.vector.tensor_copy(kT_sb[:32, h, n0:n0 + 384], kp[:32, :])

            # -- v = x @ Wv -> Vx [j, jb, h, d] + ones column --
            for jb in range(NJB):
                vp = nps.tile([128, 384], f32, tag="nmm", bufs=2)
                for kt in range(2):
                    nc.tensor.matmul(vp[:, 0:INNER], lhsT=xT_sb[:, kt, jb * 128:(jb + 1) * 128],
                                     rhs=wn_sb[:, kt, 2 * INNER:3 * INNER],
                                     start=(kt == 0), stop=(kt == 1))
                nc.vector.tensor_copy(Vx_sb[:, jb, :, 0:D],
                                      vp[:, 0:INNER].rearrange("p (h d) -> p h d", h=H))
            nc.vector.memset(Vx_sb[:, :, :, D:D + 1], 1.0)

            # -- q^T = (Wq*scale)^T @ xq^T : per-head [d, i] --
            for h in range(H):
                qp = nps.tile([128, 384], f32, tag="nmm", bufs=2)
                for kt in range(2):
                    nc.tensor.matmul(qp[:32, 0:NI], lhsT=wn_sb[:, kt, h * D:(h + 1) * D],
                                     rhs=xqT_sb[:, kt, :],
                                     start=(kt == 0), stop=(kt == 1))
                nc.vector.tensor_copy(qT_sb[:32, h, :], qp[:32, 0:NI])

            # -- g = xq @ Wg + bg ; sig = sigmoid(g) --
            gp = nps.tile([128, 384], f32, tag="nmm", bufs=2)
            for kt in range(2):
                nc.tensor.matmul(gp[:NI, 0:INNER], lhsT=xqT_sb[:, kt, :],
                                 rhs=wn_sb[:, kt, 3 * INNER:4 * INNER],
                                 start=(kt == 0), stop=(kt == 1))
            gt = npool.tile([NI, INNER], f32)
            nc.vector.tensor_add(gt[:], gp[:NI, 0:INNER], bg_sb[:NI, :])
            nc.scalar.activation(sig_sb[:NI, :], gt[:],
                                 mybir.ActivationFunctionType.Sigmoid)


        # ================= pair path (jb loop) =================
        with tc.tile_pool(name="sim_ps", bufs=1, space="PSUM") as simp, \
             tc.tile_pool(name="att_ps", bufs=2, space="PSUM") as attp:
            for jb in range(2, NJB):
                stageA(jb)
                stageB(jb - 2)
            stageB(NJB - 2)
            stageB(NJB - 1)

        pctx.close()   # release TP/tsq SBUF + dots/ss PSUM before finalize
        # ---- finalize ----
        if True:
            with tc.tile_pool(name="fin", bufs=1) as fin, \
                 tc.tile_pool(name="fin_ps", bufs=2, space="PSUM") as finp:
                den_r = fin.tile([NI, H], f32)
                for h in range(H):
                    nc.vector.reciprocal(den_r[:, h:h + 1], att_acc[:, h, D:D + 1])
                att_f = fin.tile([NI, INNER], f32)
                for h in range(H):
                    nc.vector.tensor_scalar_mul(att_f[:, h * D:(h + 1) * D],
                                                att_acc[:, h, 0:D],
                                                den_r[:, h:h + 1])
                gated = fin.tile([NI, INNER], f32)
                nc.vector.tensor_mul(gated[:], att_f[:], sig_sb[:NI, :])

                gT_sb = fin.tile([128, 2, NI], bf16)
                for kt in range(2):
                    tp = finp.tile([128, NI], f32, tag="gpose")
                    nc.tensor.transpose(tp[:], gated[:, kt * 128:(kt + 1) * 128],
                                        ident[:NI, :NI])
                    nc.vector.tensor_copy(gT_sb[:, kt, :], tp[:])

                y_ps = finp.tile([NI, ND], f32, tag="ymm")
                for kt in range(2):
                    nc.tensor.matmul(y_ps[:], lhsT=gT_sb[:, kt, :],
                                     rhs=wout_sb[:, kt, :],
                                     start=(kt == 0), stop=(kt == 1))
                y_sb = fin.tile([NI, ND], f32)
                nc.vector.tensor_add(y_sb[:], y_ps[:], bout_sb[:NI, :])
                nc.sync.dma_start(out=y_out, in_=y_sb[:])

    return nc


def host_prep(inputs, NI=96, n_cores=8):
    """Slice/fold FULL inputs into per-core in_maps."""
    import ml_dtypes
    bf = ml_dtypes.bfloat16
    node_feats = np.asarray(inputs["node_feats"])[0]      # [N, ND]
    pair_feats = np.asarray(inputs["pair_feats"])[0]      # [N, N, C]
    mask = np.asarray(inputs["mask"])[0]                  # [N, N] bool
    lnw = np.asarray(inputs["ln_node_w"]).reshape(1, ND)
    lnb = np.asarray(inputs["ln_node_b"]).reshape(1, ND)
    lpw = np.asarray(inputs["ln_pair_w"])                 # [C]
    lpb = np.asarray(inputs["ln_pair_b"])                 # [C]
    w_qkv = np.asarray(inputs["w_qkv"])                   # [ND, 3*INNER]
    w_g = np.asarray(inputs["w_g"])                       # [ND, INNER]
    b_g = np.asarray(inputs["b_g"]).reshape(1, INNER)
    w_bias = np.asarray(inputs["w_bias"])                 # [C, H]
    w_out = np.asarray(inputs["w_out"])                   # [INNER, ND]
    b_out = np.asarray(inputs["b_out"]).reshape(1, ND)

    Wp = lpw[:, None] * w_bias                            # [C, H]
    s_h = Wp.sum(0)
    t_h = (lpb[:, None] * w_bias).sum(0)
    wext = np.zeros((C, 16), np.float32)
    wext[:, 0:H] = Wp - s_h[None, :] / C                  # dot' = dot - s_h*mu
    wext[:, 8] = 1.0 / C
    wext = wext.astype(bf)

    scale = D ** -0.5
    # ln_node_w folded into the node-side weights (rows are feature-indexed)
    wnode = (lnw.reshape(ND, 1) *
             np.concatenate([w_qkv[:, 0:INNER] * scale,
                             w_qkv[:, INNER:2 * INNER],
                             w_qkv[:, 2 * INNER:3 * INNER],
                             w_g], axis=1)).astype(bf)
    woutb = w_out.astype(bf)
    threp = np.ascontiguousarray(
        np.broadcast_to(t_h[None, :, None], (1, H, NI))).astype(np.float32)

    shared = dict(node=node_feats.astype(np.float32), wext=wext, wnode=wnode,
                  wout=woutb, lnw=lnw.astype(np.float32), lnb=lnb.astype(np.float32),
                  bg=b_g.astype(np.float32), bout=b_out.astype(np.float32),
                  threp=threp)
    in_maps = []
    for c in range(n_cores):
        i0 = c * NI
        slab = pair_feats[i0:i0 + NI]                     # [NI, N, C] f32
        pairT = slab.transpose(2, 0, 1).reshape(C, NI, NJB, 128) \
                    .transpose(0, 2, 1, 3).astype(bf)     # [C, NJB, NI, 128]
        # mask pre-transposed to the kernel's [j%128, jb, i] layout
        m01 = mask[i0:i0 + NI].T.reshape(NJB, 128, NI) \
                  .transpose(1, 0, 2).astype(bf)
        in_maps.append(dict(
            pairT=np.ascontiguousarray(pairT),
            m01=np.ascontiguousarray(m01),
            nodeq=np.ascontiguousarray(node_feats[i0:i0 + NI]).astype(np.float32),
            **shared))
    return in_maps


def split_sync_waits(nc, limit=1):
    """Walrus (this container's neuronxcc) rejects instructions carrying more
    than `limit` sem waits. Hoist excess waits onto per-engine carrier drains
    inserted just before the offending instruction."""
    n_split = 0
    for f in nc.m.functions:
        for bb in f.blocks:
            out = []
            for inst in bb.instructions:
                si = inst.sync_info
                waits = list(si.on_wait) if si and si.on_wait else []
                if len(waits) > limit:
                    extra, keep = waits[:-limit], waits[-limit:]
                    for ci in range(0, len(extra), limit):
                        chunk = extra[ci:ci+limit]
                        nd = mybir.InstDrain(name=f"{inst.name}-wsplit{ci}", ins=[], outs=[])
                        nd.engine = inst.engine
                        nd.sync_info = mybir.SyncInfo(on_wait=chunk, on_update=[])
                        out.append(nd)
                        n_split += 1
                    si.on_wait = keep
                out.append(inst)
            bb.instructions = out
    return n_split


_CACHED = {}


def kernel(**inputs):
    """Full-input entry point: shards over 8 NeuronCores, returns full output."""
    NC_CORES = 8
    NI = N // NC_CORES
    from concourse.bass_utils import run_bass_kernel_spmd

    in_maps = host_prep(inputs, NI=NI, n_cores=NC_CORES)
    trivial_lnb = not np.any(np.asarray(inputs["ln_node_b"]))
    key = ("nc", trivial_lnb)
    if key not in _CACHED:
        nc = build_nc(NI=NI, n_cores=NC_CORES, trivial_lnb=trivial_lnb)
        split_sync_waits(nc)
        _CACHED[key] = nc
    res = run_bass_kernel_spmd(_CACHED[key], in_maps, list(range(NC_CORES)))
    y = np.concatenate([res.results[c]["y"] for c in range(NC_CORES)], axis=0)
    return y[None].astype(np.float32)


# revision 23
# speedup vs baseline: 1.1204x; 1.0690x over previous
"""NodeAttention Trainium2 kernel (per-core program, SPMD over 8 cores).

Strategy (per core, i-block of NI=96 query rows):
- host pre-arranges the core's pair slice as pairT [c, jb, i, j] bf16 so the
  device does one contiguous full-bandwidth DMA per j-block (24.6KB per
  partition) -- no SWDGE gather, no SBUF transposes.
- pair pools (SBUF TP buffers + dots/ss PSUM banks) open BEFORE the node
  section so TP prefetch and the jb0 projection work overlap node compute.
- pair LN + bias projection folded into a [128 chan -> 9] matmul:
  cols 0-7: lnw*w_bias - s_h/C (so dot' = dot - s_h*mu), col 8: 1/C (mean).
  bias_h = r*dot'_h + t_h with r = rsqrt(var+eps), var = sumsq/C - mu^2,
  sumsq via ones-matmul on squared tiles (squares split scalar/vector;
  scalar groups SQUARE ops to avoid activation-table thrash).
- t_h enters through an augmented 33rd contraction row of the q/k matmul
  (kT row 32 = 1, qT row 32 = t_h), so logits = sim' + r*dot' in one add.
- softmax without max-subtraction (logits bounded); normalizer via a ones
  column appended to V, so no partition reductions are needed.
"""
import numpy as np
from contextlib import ExitStack

import concourse.bass as bass
import concourse.tile as tile
from concourse import mybir
from concourse.masks import make_identity

f32 = mybir.dt.float32
bf16 = mybir.dt.bfloat16
u8 = mybir.dt.uint8

N = 768          # sequence length (j axis, also full i)
C = 128          # pair channels
H = 8            # heads
D = 32           # head dim
INNER = 256      # H*D
ND = 256         # node dim
NJB = N // 128   # 6 j-blocks
EPS = 1e-5


def _bcast_h(ap2d: bass.AP, h: int) -> bass.AP:
    """[P, F] -> [P, h, F] with step-0 broadcast over the middle dim."""
    ap = list(ap2d.ap)
    assert len(ap) == 2
    return bass.AP(ap2d.tensor, ap2d.offset, [ap[0], [0, h], ap[1]])


def _swap_hi(ap3: bass.AP, i_off: int, i_cnt: int) -> bass.AP:
    """logits [P, H, NI] tile -> iteration [P, i_cnt, H] at i offset."""
    p, hdim, idim = ap3.ap
    return bass.AP(ap3.tensor, ap3.offset + i_off * idim[0],
                   [p, [idim[0], i_cnt], hdim])


def _bcast_last(ap2d: bass.AP, i_off: int, i_cnt: int, h: int) -> bass.AP:
    """r [P, NI] -> iteration [P, i_cnt, h(step0)] at i offset."""
    p, f = ap2d.ap
    return bass.AP(ap2d.tensor, ap2d.offset + i_off * f[0],
                   [p, [f[0], i_cnt], [0, h]])


def build_nc(NI=96, n_cores=8, upto='full', trivial_lnb=True):
    nc = bass.Bass("TRN2", target_bir_lowering=False, debug=False,
                   num_devices=n_cores)
    # pair slice pre-transposed on host: pairT[c, jb, i, j] bf16
    pairT = nc.dram_tensor("pairT", [C, NJB, NI, 128], bf16,
                           kind="ExternalInput").ap()
    node = nc.dram_tensor("node", [N, ND], f32, kind="ExternalInput").ap()
    nodeq = nc.dram_tensor("nodeq", [NI, ND], f32, kind="ExternalInput").ap()
    m01 = nc.dram_tensor("m01", [128, NJB, NI], bf16, kind="ExternalInput").ap()
    wext = nc.dram_tensor("wext", [C, 16], bf16, kind="ExternalInput").ap()
    # wnode cols: [Wq*scale | Wk | Wv | Wg]
    wnode = nc.dram_tensor("wnode", [ND, 4 * INNER], bf16, kind="ExternalInput").ap()
    wout = nc.dram_tensor("wout", [INNER, ND], bf16, kind="ExternalInput").ap()
    lnw = nc.dram_tensor("lnw", [1, ND], f32, kind="ExternalInput").ap()
    lnb = nc.dram_tensor("lnb", [1, ND], f32, kind="ExternalInput").ap()
    bg = nc.dram_tensor("bg", [1, INNER], f32, kind="ExternalInput").ap()
    bout = nc.dram_tensor("bout", [1, ND], f32, kind="ExternalInput").ap()
    # t_h replicated over i (augmented q row)
    threp = nc.dram_tensor("threp", [1, H, NI], f32, kind="ExternalInput").ap()
    y_out = nc.dram_tensor("y", [NI, ND], f32, kind="ExternalOutput").ap()
    dbg = nc.dram_tensor("dbg", [128, 2048], f32, kind="ExternalOutput").ap() \
        if upto != 'full' else None

    NIT = (NI + 31) // 32          # dots psum tiles per jb

    with tile.TileContext(nc) as tc, ExitStack() as ctx:
        const = ctx.enter_context(tc.tile_pool(name="const", bufs=1))
        persist = ctx.enter_context(tc.tile_pool(name="persist", bufs=1))
        # pair-path pools, opened early so TP prefetch + jb0 dots/ss overlap
        # the node section (PSUM: dots 3 banks + ss 1 bank; node uses 4).
        jw = ctx.enter_context(tc.tile_pool(name="jwork", bufs=2))
        accp = ctx.enter_context(tc.tile_pool(name="att_acc", bufs=1))
        pctx = ctx.enter_context(ExitStack())
        dps = pctx.enter_context(tc.tile_pool(name="dots_ps", bufs=NIT, space="PSUM"))
        ssp = pctx.enter_context(tc.tile_pool(name="ss_ps", bufs=1, space="PSUM"))
        tpp = pctx.enter_context(tc.tile_pool(name="tp", bufs=4))
        sqp = pctx.enter_context(tc.tile_pool(name="tpsq", bufs=3))

        # ---- constants ----
        wext_sb = const.tile([C, 16], bf16)
        nc.sync.dma_start(out=wext_sb[:], in_=wext)
        ones_sb = const.tile([C, 1], bf16)
        nc.vector.memset(ones_sb[:], 1.0)
        ident = const.tile([128, 128], f32)
        make_identity(nc, ident[:])
        eps_sb = const.tile([128, 1], f32)
        nc.vector.memset(eps_sb[:], EPS)
        # broadcast loads (replicate along partitions via step-0 DMA)
        def bload(name, src, cols, dtype=f32):
            t = const.tile([128, cols], dtype)
            src_b = bass.AP(src.tensor, src.offset, [[0, 128]] + list(src.ap)[1:])
            nc.gpsimd.dma_start(out=t[:], in_=src_b)
            return t
        lnb_sb = bload("lnb", lnb, ND)
        bg_sb = bload("bg", bg, INNER)
        bout_sb = bload("bout", bout, ND)
        # node-side weights, feat-major tiles [feat%128, feat//128, cols]
        wn_sb = const.tile([128, 2, 4 * INNER], bf16)
        wout_sb = const.tile([128, 2, ND], bf16)

        # ---- persistent node-derived tensors (33rd row = t_h injection) ----
        kT_sb = persist.tile([33, H, N], bf16)        # k^T [d, h, j]; row 32 = 1
        qT_sb = persist.tile([33, H, NI], bf16)       # q^T [d, h, i]; row 32 = t_h
        Vx_sb = persist.tile([128, NJB, H, D + 1], bf16)  # v in [j, jb, h, d|1]
        m01T_sb = persist.tile([128, NJB, NI], bf16)  # mask^T in [j, jb, i]
        sig_sb = persist.tile([max(NI, 1), INNER], f32)  # sigmoid(g) [i, inner]

        nc.vector.memset(kT_sb[32:33, :, :], 1.0)
        nc.gpsimd.dma_start(out=qT_sb[32:33, :, :], in_=threp)  # f32 -> bf16 cast

        att_acc = accp.tile([NI, H, D + 1], f32)
        nc.vector.memset(att_acc[:], 0.0)

        # ---- pair-path stages (A: node-independent; B: needs k/q/V/mask) ----
        A = {}

        def stageA(jb):
            TP = TPs[jb]
            dots_tiles = []
            for t in range(NIT):
                dt_ = dps.tile([128, 32, 16], f32, tag="dots", name=f"dt{jb}_{t}")
                dots_tiles.append(dt_)
            for i in range(NI):
                nc.tensor.matmul(dots_tiles[i // 32][:, i % 32, 0:9],
                                 lhsT=TP[:, i, :], rhs=wext_sb[:, 0:9])
            # sumsq via squared tiles: chunk 0 scalar, 1-2 vector
            ss_ps = ssp.tile([128, NI], f32, tag="ss", name=f"ss{jb}")
            for t in range(NIT):
                i0 = t * 32
                tsq = sqp.tile([C, 32, 128], bf16, tag="tpsq", name=f"tsq{jb}_{t}")
                src = TP[:, i0:i0 + 32, :]
                if t == 0:
                    nc.scalar.square(tsq[:], src)
                else:
                    nc.vector.tensor_mul(tsq[:], src, src)
                for il in range(32):
                    nc.tensor.matmul(ss_ps[:, i0 + il:i0 + il + 1],
                                     lhsT=tsq[:, il, :], rhs=ones_sb[:])
            # early PSUM->SBUF copy of dots so the banks recycle immediately
            dots_sb = jw.tile([128, NI, 9], f32, tag="dots_sb", bufs=3,
                              name=f"dsb{jb}")
            for t in range(NIT):
                nc.vector.tensor_copy(dots_sb[:, t * 32:(t + 1) * 32, :],
                                      dots_tiles[t][:, :, 0:9])
            # stats: var = ss/C - mu^2 ; r = 1/sqrt(var + eps)
            m2 = jw.tile([128, NI], f32, tag="m2", bufs=3, name=f"m2_{jb}")
            nc.scalar.square(m2[:], dots_sb[:, :, 8])
            var = jw.tile([128, NI], f32, tag="var", bufs=3, name=f"var{jb}")
            nc.vector.scalar_tensor_tensor(
                out=var[:], in0=ss_ps[:], scalar=1.0 / C, in1=m2[:],
                op0=mybir.AluOpType.mult, op1=mybir.AluOpType.subtract)
            sdp = jw.tile([128, NI], f32, tag="sdp", bufs=3, name=f"sdp{jb}")
            nc.scalar.activation(sdp[:], var[:],
                                 mybir.ActivationFunctionType.Sqrt,
                                 bias=eps_sb[:])
            r = jw.tile([128, NI], f32, tag="r", bufs=3, name=f"r{jb}")
            nc.vector.reciprocal(r[:], sdp[:])
            A[jb] = (dots_sb, r)

        def stageB(jb):
            dots_sb, r = A.pop(jb)
            sim_ps = simp.tile([128, H, 128], f32, tag="sim", name=f"sim{jb}")
            for h in range(H):
                nc.tensor.matmul(
                    sim_ps[:, h, 0:NI],
                    lhsT=kT_sb[:, h, jb * 128:(jb + 1) * 128],
                    rhs=qT_sb[:, h, :])
            # logits = sim' + r*dots' ; E = exp * mask
            logits = jw.tile([128, H, NI], f32, tag="logits", bufs=2, name=f"lg{jb}")
            nc.vector.tensor_tensor(
                out=_swap_hi(logits[:], 0, NI),
                in0=dots_sb[:, :, 0:8],
                in1=_bcast_last(r[:], 0, NI, H),
                op=mybir.AluOpType.mult)
            nc.vector.tensor_add(logits[:], logits[:], sim_ps[:, :, 0:NI])
            E = jw.tile([128, H, NI], bf16, tag="E", bufs=2, name=f"E{jb}")
            nc.scalar.activation(E[:], logits[:],
                                 mybir.ActivationFunctionType.Exp)
            nc.vector.tensor_mul(E[:], E[:], _bcast_h(m01T_sb[:, jb, :], H))
            # attn @ [V|1]; accumulate across jb in SBUF
            att_ps = attp.tile([NI, H, D + 1], f32, tag="attjb", name=f"at{jb}")
            for h in range(H):
                nc.tensor.matmul(att_ps[:, h, :], lhsT=E[:, h, :],
                                 rhs=Vx_sb[:, jb, h, :])
            nc.vector.tensor_add(att_acc[:], att_acc[:], att_ps[:])

        with tc.tile_pool(name="nodep", bufs=1) as npool, \
             tc.tile_pool(name="node_ps", bufs=1, space="PSUM") as nps:
            # -- load + LN node (full, for k/v) and nodeq (for q/g) --
            x_all = npool.tile([128, N // 128, ND], f32)
            stats = npool.tile([128, 6], f32)
            mv = npool.tile([128, 2], f32)
            sd = npool.tile([128, 1], f32)
            rln = npool.tile([128, 1], f32)

            def layernorm_tile(xt, nrows):
                nc.vector.bn_stats(out=stats[:nrows, :], in_=xt)
                nc.vector.bn_aggr(out=mv[:nrows, :], in_=stats[:nrows, :])
                nc.scalar.activation(sd[:nrows, :], mv[:nrows, 1:2],
                                     mybir.ActivationFunctionType.Sqrt,
                                     bias=eps_sb[:nrows, :])
                nc.vector.reciprocal(rln[:nrows, :], sd[:nrows, :])
                nc.vector.tensor_scalar(out=xt, in0=xt,
                                        scalar1=mv[:nrows, 0:1],
                                        scalar2=rln[:nrows, :],
                                        op0=mybir.AluOpType.subtract,
                                        op1=mybir.AluOpType.mult)
                if not trivial_lnb:
                    nc.vector.tensor_add(xt, xt, lnb_sb[:nrows, :])

            # sync HWDGE queue is FIFO: node inputs first (small, needed
            # first), then weights, then the 12MB TP bulk.
            nc.sync.dma_start(out=x_all[:],
                              in_=node.rearrange("(t p) c -> p t c", p=128))
            xq = npool.tile([NI, ND], f32)
            nc.sync.dma_start(out=xq[:], in_=nodeq)
            nc.sync.dma_start(out=m01T_sb[:], in_=m01)
            nc.sync.dma_start(out=wn_sb[:],
                              in_=wnode.rearrange("(kt p) c -> p kt c", p=128))
            nc.sync.dma_start(out=wout_sb[:],
                              in_=wout.rearrange("(kt p) c -> p kt c", p=128))
            TPs = []
            for jb in range(NJB):
                TP = tpp.tile([C, NI, 128], bf16, tag="tp", name=f"TP{jb}")
                nc.sync.dma_start(out=TP[:], in_=pairT[:, jb, :, :])
                TPs.append(TP)

            for t in range(N // 128):
                layernorm_tile(x_all[:, t, :], 128)
            layernorm_tile(xq[:], NI)

            # jb0/jb1 projection work is node-independent: emit it here so
            # PE/vector/scalar have pair work queued while node copies drain.
            stageA(0)
            stageA(1)

            # -- transposes: xT [feat, j], xqT [feat, i] --
            xT_sb = npool.tile([128, 2, N], bf16)
            xqT_sb = npool.tile([128, 2, NI], bf16)
            for t in range(N // 128):
                tp = nps.tile([128, 2, 128], f32, tag="xpose", bufs=1)
                for kt in range(2):
                    nc.tensor.transpose(tp[:, kt, :], x_all[:, t, kt * 128:(kt + 1) * 128], ident[:])
                nc.vector.tensor_copy(xT_sb[:, :, t * 128:(t + 1) * 128], tp[:])
            tp = nps.tile([128, 2, 128], f32, tag="xpose", bufs=1)
            for kt in range(2):
                nc.tensor.transpose(tp[:, kt, :NI], xq[:, kt * 128:(kt + 1) * 128], ident[:NI, :NI])
            nc.vector.tensor_copy(xqT_sb[:, :, :], tp[:, :, :NI])

            # -- k^T = Wk^T @ x^T : per-head [d, j] at partition base 0 --
            for h in range(H):
                for n0 in range(0, N, 384):
                    kp = nps.tile([128, 384], f32, tag="nmm", bufs=2)
                    for kt in range(2):
                        nc.tensor.matmul(
                            kp[:32, :],
                            lhsT=wn_sb[:, kt, INNER + h * D:INNER + (h + 1) * D],
                            rhs=xT_sb[:, kt, n0:n0 + 384],
                            start=(kt == 0), stop=(kt == 1))
                    nc.vector.tensor_copy(kT_sb[:32, h, n0:n0 + 384], kp[:32, :])

            # -- v = x @ Wv -> Vx [j, jb, h, d] + ones column --
            for jb in range(NJB):
                vp = nps.tile([128, 384], f32, tag="nmm", bufs=2)
                for kt in range(2):
                    nc.tensor.matmul(vp[:, 0:INNER], lhsT=xT_sb[:, kt, jb * 128:(jb + 1) * 128],
                                     rhs=wn_sb[:, kt, 2 * INNER:3 * INNER],
                                     start=(kt == 0), stop=(kt == 1))
                nc.vector.tensor_copy(Vx_sb[:, jb, :, 0:D],
                                      vp[:, 0:INNER].rearrange("p (h d) -> p h d", h=H))
            nc.vector.memset(Vx_sb[:, :, :, D:D + 1], 1.0)

            # -- q^T = (Wq*scale)^T @ xq^T : per-head [d, i] --
            for h in range(H):
                qp = nps.tile([128, 384], f32, tag="nmm", bufs=2)
                for kt in range(2):
                    nc.tensor.matmul(qp[:32, 0:NI], lhsT=wn_sb[:, kt, h * D:(h + 1) * D],
                                     rhs=xqT_sb[:, kt, :],
                                     start=(kt == 0), stop=(kt == 1))
                nc.vector.tensor_copy(qT_sb[:32, h, :], qp[:32, 0:NI])

            # -- g = xq @ Wg + bg ; sig = sigmoid(g) --
            gp = nps.tile([128, 384], f32, tag="nmm", bufs=2)
            for kt in range(2):
                nc.tensor.matmul(gp[:NI, 0:INNER], lhsT=xqT_sb[:, kt, :],
                                 rhs=wn_sb[:, kt, 3 * INNER:4 * INNER],
                                 start=(kt == 0), stop=(kt == 1))
            gt = npool.tile([NI, INNER], f32)
            nc.vector.tensor_add(gt[:], gp[:NI, 0:INNER], bg_sb[:NI, :])
            nc.scalar.activation(sig_sb[:NI, :], gt[:],
                                 mybir.ActivationFunctionType.Sigmoid)


        # ================= pair path (jb loop) =================
        with tc.tile_pool(name="sim_ps", bufs=1, space="PSUM") as simp, \
             tc.tile_pool(name="att_ps", bufs=2, space="PSUM") as attp:
            for jb in range(2, NJB):
                stageA(jb)
                stageB(jb - 2)
            stageB(NJB - 2)
            stageB(NJB - 1)

        pctx.close()   # release TP/tsq SBUF + dots/ss PSUM before finalize
        # ---- finalize ----
        if True:
            with tc.tile_pool(name="fin", bufs=1) as fin, \
                 tc.tile_pool(name="fin_ps", bufs=2, space="PSUM") as finp:
                den_r = fin.tile([NI, H], f32)
                for h in range(H):
                    nc.vector.reciprocal(den_r[:, h:h + 1], att_acc[:, h, D:D + 1])
                att_f = fin.tile([NI, INNER], f32)
                for h in range(H):
                    nc.vector.tensor_scalar_mul(att_f[:, h * D:(h + 1) * D],
                                                att_acc[:, h, 0:D],
                                                den_r[:, h:h + 1])
                gated = fin.tile([NI, INNER], f32)
                nc.vector.tensor_mul(gated[:], att_f[:], sig_sb[:NI, :])

                gT_sb = fin.tile([128, 2, NI], bf16)
                for kt in range(2):
                    tp = finp.tile([128, NI], f32, tag="gpose")
                    nc.tensor.transpose(tp[:], gated[:, kt * 128:(kt + 1) * 128],
                                        ident[:NI, :NI])
                    nc.vector.tensor_copy(gT_sb[:, kt, :], tp[:])

                y_ps = finp.tile([NI, ND], f32, tag="ymm")
                for kt in range(2):
                    nc.tensor.matmul(y_ps[:], lhsT=gT_sb[:, kt, :],
                                     rhs=wout_sb[:, kt, :],
                                     start=(kt == 0), stop=(kt == 1))
                y_sb = fin.tile([NI, ND], f32)
                nc.vector.tensor_add(y_sb[:], y_ps[:], bout_sb[:NI, :])
                nc.sync.dma_start(out=y_out, in_=y_sb[:])

    return nc


def host_prep(inputs, NI=96, n_cores=8):
    """Slice/fold FULL inputs into per-core in_maps."""
    import ml_dtypes
    bf = ml_dtypes.bfloat16
    node_feats = np.asarray(inputs["node_feats"])[0]      # [N, ND]
    pair_feats = np.asarray(inputs["pair_feats"])[0]      # [N, N, C]
    mask = np.asarray(inputs["mask"])[0]                  # [N, N] bool
    lnw = np.asarray(inputs["ln_node_w"]).reshape(1, ND)
    lnb = np.asarray(inputs["ln_node_b"]).reshape(1, ND)
    lpw = np.asarray(inputs["ln_pair_w"])                 # [C]
    lpb = np.asarray(inputs["ln_pair_b"])                 # [C]
    w_qkv = np.asarray(inputs["w_qkv"])                   # [ND, 3*INNER]
    w_g = np.asarray(inputs["w_g"])                       # [ND, INNER]
    b_g = np.asarray(inputs["b_g"]).reshape(1, INNER)
    w_bias = np.asarray(inputs["w_bias"])                 # [C, H]
    w_out = np.asarray(inputs["w_out"])                   # [INNER, ND]
    b_out = np.asarray(inputs["b_out"]).reshape(1, ND)

    Wp = lpw[:, None] * w_bias                            # [C, H]
    s_h = Wp.sum(0)
    t_h = (lpb[:, None] * w_bias).sum(0)
    wext = np.zeros((C, 16), np.float32)
    wext[:, 0:H] = Wp - s_h[None, :] / C                  # dot' = dot - s_h*mu
    wext[:, 8] = 1.0 / C
    wext = wext.astype(bf)

    scale = D ** -0.5
    # ln_node_w folded into the node-side weights (rows are feature-indexed)
    wnode = (lnw.reshape(ND, 1) *
             np.concatenate([w_qkv[:, 0:INNER] * scale,
                             w_qkv[:, INNER:2 * INNER],
                             w_qkv[:, 2 * INNER:3 * INNER],
                             w_g], axis=1)).astype(bf)
    woutb = w_out.astype(bf)
    threp = np.ascontiguousarray(
        np.broadcast_to(t_h[None, :, None], (1, H, NI))).astype(np.float32)

    shared = dict(node=node_feats.astype(np.float32), wext=wext, wnode=wnode,
                  wout=woutb, lnw=lnw.astype(np.float32), lnb=lnb.astype(np.float32),
                  bg=b_g.astype(np.float32), bout=b_out.astype(np.float32),
                  threp=threp)
    in_maps = []
    for c in range(n_cores):
        i0 = c * NI
        slab = pair_feats[i0:i0 + NI]                     # [NI, N, C] f32
        pairT = slab.transpose(2, 0, 1).reshape(C, NI, NJB, 128) \
                    .transpose(0, 2, 1, 3).astype(bf)     # [C, NJB, NI, 128]
        # mask pre-transposed to the kernel's [j%128, jb, i] layout
        m01 = mask[i0:i0 + NI].T.reshape(NJB, 128, NI) \
                  .transpose(1, 0, 2).astype(bf)
        in_maps.append(dict(
            pairT=np.ascontiguousarray(pairT),
            m01=np.ascontiguousarray(m01),
            nodeq=np.ascontiguousarray(node_feats[i0:i0 + NI]).astype(np.float32),
            **shared))
    return in_maps


def split_sync_waits(nc, limit=1):
    """Walrus (this container's neuronxcc) rejects instructions carrying more
    than `limit` sem waits. Hoist excess waits onto per-engine carrier drains
    inserted just before the offending instruction."""
    n_split = 0
    for f in nc.m.functions:
        for bb in f.blocks:
            out = []
            for inst in bb.instructions:
                si = inst.sync_info
                waits = list(si.on_wait) if si and si.on_wait else []
                if len(waits) > limit:
                    extra, keep = waits[:-limit], waits[-limit:]
                    for ci in range(0, len(extra), limit):
                        chunk = extra[ci:ci+limit]
                        nd = mybir.InstDrain(name=f"{inst.name}-wsplit{ci}", ins=[], outs=[])
                        nd.engine = inst.engine
                        nd.sync_info = mybir.SyncInfo(on_wait=chunk, on_update=[])
                        out.append(nd)
                        n_split += 1
                    si.on_wait = keep
                out.append(inst)
            bb.instructions = out
    return n_split


_CACHED = {}


def kernel(**inputs):
    """Full-input entry point: shards over 8 NeuronCores, returns full output."""
    NC_CORES = 8
    NI = N // NC_CORES
    from concourse.bass_utils import run_bass_kernel_spmd

    in_maps = host_prep(inputs, NI=NI, n_cores=NC_CORES)
    trivial_lnb = not np.any(np.asarray(inputs["ln_node_b"]))
    key = ("nc", trivial_lnb)
    if key not in _CACHED:
        nc = build_nc(NI=NI, n_cores=NC_CORES, trivial_lnb=trivial_lnb)
        split_sync_waits(nc)
        _CACHED[key] = nc
    res = run_bass_kernel_spmd(_CACHED[key], in_maps, list(range(NC_CORES)))
    y = np.concatenate([res.results[c]["y"] for c in range(NC_CORES)], axis=0)
    return y[None].astype(np.float32)
